# revision 36
# baseline (speedup 1.0000x reference)
"""Trainium2 Bass kernel for BiLinearSigmoidAttention (v2).

Reference math (per batch b, with L = length[b]):
    qn = l2norm(query), cn = l2norm(context)
    raw[q,k] = qn[q] . cn[k]            (masked: k >= L -> -1e30)
    sig = sigmoid(raw)
    den[q] = max(sum_k sig[q,k], 1)
    scores[q,k] = sig[q,k] / den[q]     (rows q >= L zeroed)
    att[q,:] = sum_k scores[q,k] * context[k,:]
    out = concat([qn, att], -1)
returns (out [B,S,2D], scores [B,S,S])

Device mapping (8 NeuronCores, pure data parallel over B=32 -> 4 per core):
  - all PE transposes stream a bf16 identity (1 cycle/row); data side is
    f32r (q/c) or bf16 (sig).
  - qT/cT are cast to bf16 during PSUM eviction -> mm1 runs bf16 weights
    x bf16 moving (full rate, half-cost LDWEIGHTS).
  - sigmoid eviction writes sg as bf16: mm2 weights bf16, scoresT
    transposes bf16, half SBUF.
  - denominator: free-dim reduce over the transposed scores PSUM (DVE)
    instead of ones-column matmuls -> no PE weight reloads for dn.
  - element-wise work split across ACT / DVE / GpSimd(Pool).
  - loads issue on the sync DGE queue, stores on the gpsimd queue.
"""

import numpy as np
import ml_dtypes

import concourse.bacc as bacc
import concourse.mybir as mybir
import concourse.tile as tile
from concourse.bass_utils import run_bass_kernel_spmd

B, S, D = 32, 1024, 512
NCORES = 8
BPC = B // NCORES          # batches per core
P = 128                    # partitions
NT = S // P                # 8 s-tiles
ND = D // P                # 4 d-chunks
HT = NT // 2               # s-tiles per half
NEG = np.float32(-1e30)

F32 = mybir.dt.float32
F32R = mybir.dt.float32r
BF16 = mybir.dt.bfloat16
AF = mybir.ActivationFunctionType
AX = mybir.AxisListType


def _r(ap):
    return ap.bitcast(F32R)


def _f(ap):
    return ap.bitcast(F32)


def build_kernel():
    nc = bacc.Bacc("TRN2", target_bir_lowering=False, debug=False)

    q_d = nc.dram_tensor("query", [BPC, S, D], F32R, kind="ExternalInput")
    c_d = nc.dram_tensor("context", [BPC, S, D], F32R, kind="ExternalInput")
    # keybias[b, p, kt] = 0 if kt*P+p < L else -1e30
    kb_d = nc.dram_tensor("keybias", [BPC, P, NT], F32, kind="ExternalInput")
    # qmask[b, p, qb] = 1 if qb*P+p < L else 0
    qm_d = nc.dram_tensor("qmask", [BPC, P, NT], F32, kind="ExternalInput")
    id_d = nc.dram_tensor("identity16", [P, P], BF16, kind="ExternalInput")
    idr_d = nc.dram_tensor("identity_r", [P, P], F32R, kind="ExternalInput")
    out_d = nc.dram_tensor("out", [BPC, S, 2 * D], F32, kind="ExternalOutput")
    sc_d = nc.dram_tensor("scores", [BPC, S, S], F32, kind="ExternalOutput")

    with tile.TileContext(nc) as tc:
        _body(tc, q_d, c_d, kb_d, qm_d, id_d, idr_d, out_d, sc_d)
    nc.compile()
    return nc


def _body(tc, q_d, c_d, kb_d, qm_d, id_d, idr_d, out_d, sc_d):
    nc = tc.nc
    from contextlib import ExitStack

    ctx = ExitStack()
    with ctx:
        const = ctx.enter_context(tc.tile_pool(name="const", bufs=1))
        qpool = ctx.enter_context(tc.tile_pool(name="q", bufs=2))
        cpool = ctx.enter_context(tc.tile_pool(name="c", bufs=2))
        tpool = ctx.enter_context(tc.tile_pool(name="t", bufs=1))
        sgpool = ctx.enter_context(tc.tile_pool(name="sg", bufs=1))
        mpool = ctx.enter_context(tc.tile_pool(name="m", bufs=2))
        spool = ctx.enter_context(tc.tile_pool(name="s", bufs=2))
        wpool = ctx.enter_context(tc.tile_pool(name="w", bufs=4))
        opool = ctx.enter_context(tc.tile_pool(name="o", bufs=3))
        # shared 4KB-slot pool: transpose pairs ([P,8,128] f32r) and mm1
        # accumulators ([P,2,512] f32) alternate phases -> 4 banks total
        psA = ctx.enter_context(tc.tile_pool(name="psA", bufs=2, space="PSUM"))
        ps2 = ctx.enter_context(tc.tile_pool(name="ps2", bufs=2, space="PSUM"))
        pss = ctx.enter_context(tc.tile_pool(name="pss", bufs=2, space="PSUM"))

        id16 = const.tile([P, P], BF16, tag="id16")
        idr = const.tile([P, P], F32R, tag="idr")
        nc.sync.dma_start(id16[:], id_d[:])
        nc.sync.dma_start(idr[:], idr_d[:])

        for b in range(BPC):
            # ---- loads: context first (cT transposes need only ct) ----
            qt = qpool.tile([P, NT, D], F32R, tag="qt")
            ct = cpool.tile([P, NT, D], F32R, tag="ct")
            kb = mpool.tile([P, NT], F32, tag="kb")
            qm = mpool.tile([P, NT], F32, tag="qm")
            nc.sync.dma_start(ct[:], c_d[b].rearrange("(t p) d -> p t d", p=P))
            nc.sync.dma_start(
                qt[:, 0:HT], q_d[b, 0 : S // 2].rearrange("(t p) d -> p t d", p=P)
            )
            nc.sync.dma_start(kb[:], kb_d[b])
            nc.sync.dma_start(qm[:], qm_d[b])
            nc.sync.dma_start(
                qt[:, HT:NT], q_d[b, S // 2 : S].rearrange("(t p) d -> p t d", p=P)
            )

            # ---- cT transposes (PE) with DVE evict-cast to bf16 ----
            cT = tpool.tile([P, ND, S], BF16, tag="cT")
            qT = tpool.tile([P, ND, S], BF16, tag="qT")
            for tp in range(NT // 2):
                pc = psA.tile([P, 2, ND, P], F32, tag="pt")
                for tt in range(2):
                    t = 2 * tp + tt
                    for dch in range(ND):
                        nc.tensor.transpose(
                            _r(pc[:, tt, dch]),
                            ct[:, t, dch * P : (dch + 1) * P], idr[:],
                        )
                    nc.vector.tensor_copy(
                        cT[:, :, t * P : (t + 1) * P], pc[:, tt]
                    )

            # ---- norms: ACT squares with free-dim accumulate ----
            ssq = mpool.tile([P, 2 * NT], F32, tag="ssq")
            inv = mpool.tile([P, 2 * NT], F32, tag="inv")
            for t in range(NT):
                scr = spool.tile([P, D], F32, tag="scr")
                nc.scalar.activation(
                    scr[:], ct[:, t], AF.Square,
                    accum_out=ssq[:, NT + t : NT + t + 1],
                )
            nrm = mpool.tile([P, 2 * NT], F32, tag="nrm")
            nc.scalar.activation(nrm[:, NT : 2 * NT], ssq[:, NT : 2 * NT], AF.Sqrt)
            nc.vector.reciprocal(inv[:, NT : 2 * NT], nrm[:, NT : 2 * NT])

            # q norms + qn scale + qT transposes, in halves for startup
            for h in range(2):
                t0, t1 = h * HT, (h + 1) * HT
                for t in range(t0, t1):
                    scr = spool.tile([P, D], F32, tag="scr")
                    nc.scalar.activation(
                        scr[:], _f(qt[:, t]), AF.Square,
                        accum_out=ssq[:, t : t + 1],
                    )
                nc.scalar.activation(nrm[:, t0:t1], ssq[:, t0:t1], AF.Sqrt)
                nc.vector.reciprocal(inv[:, t0:t1], nrm[:, t0:t1])
                for t in range(t0, t1):
                    nc.vector.tensor_scalar_mul(
                        qt[:, t], _f(qt[:, t]), inv[:, t : t + 1]
                    )
                nc.gpsimd.dma_start(
                    out_d[b, h * (S // 2) : (h + 1) * (S // 2), 0:D].rearrange(
                        "(t p) d -> p t d", p=P
                    ),
                    _f(qt[:, t0:t1]),
                )
                for tp in range(t0 // 2, t1 // 2):
                    pq = psA.tile([P, 2, ND, P], F32, tag="pt")
                    for tt in range(2):
                        t = 2 * tp + tt
                        for dch in range(ND):
                            nc.tensor.transpose(
                                _r(pq[:, tt, dch]),
                                qt[:, t, dch * P : (dch + 1) * P], idr[:],
                            )
                        nc.scalar.copy(
                            qT[:, :, t * P : (t + 1) * P], pq[:, tt]
                        )

            # bf16 copy of context for mm2 (bf16 weights need bf16 moving side)
            cb = cpool.tile([P, NT, D], BF16, tag="cb")
            nc.scalar.copy(cb[:, 0:HT], _f(ct[:, 0:HT]))
            nc.vector.tensor_copy(cb[:, HT:NT], _f(ct[:, HT:NT]))

            # ---- mm1 + fused sigmoid (kt outer: one [P,1024] sigmoid/kt) ----
            sg = sgpool.tile([P, NT, S], BF16, tag="sg")
            for kt in range(NT):
                acc = psA.tile([P, 2, ND, P], F32, tag="pt")
                for qc in range(2):
                    for dch in range(ND):
                        nc.tensor.matmul(
                            acc[:, qc],
                            cT[:, dch, kt * P : (kt + 1) * P],
                            qT[:, dch, qc * 512 : (qc + 1) * 512],
                            start=(dch == 0),
                            stop=(dch == ND - 1),
                        )
                # context l2-normalization folds in as the per-k scale
                nc.scalar.activation(
                    sg[:, kt], acc[:],
                    AF.Sigmoid, bias=kb[:, kt : kt + 1],
                    scale=inv[:, NT + kt : NT + kt + 1],
                )

            # ---- per q-block: att (PE), scoresT (PE), den (DVE), evicts ----
            for qb in range(NT):
                att = ps2.tile([P, D], F32, tag="att")
                for kt in range(NT):
                    nc.tensor.matmul(
                        att[:], sg[:, kt, qb * P : (qb + 1) * P], cb[:, kt],
                        start=(kt == 0), stop=(kt == NT - 1),
                    )
                pt = pss.tile([P, S], BF16, tag="pt")
                for kt in range(NT):
                    nc.tensor.transpose(
                        pt[:, kt * P : (kt + 1) * P],
                        sg[:, kt, qb * P : (qb + 1) * P],
                        id16[:],
                    )
                # w = qmask / max(sum_k sig, 1)
                den = wpool.tile([P, 1], F32, tag="den")
                nc.vector.reduce_sum(den[:], pt[:], axis=AX.X)
                w = wpool.tile([P, 1], F32, tag="w")
                nc.vector.tensor_scalar_max(w[:], den[:], 1.0)
                nc.vector.reciprocal(w[:], w[:])
                nc.vector.tensor_mul(w[:], w[:], qm[:, qb : qb + 1])

                ao = opool.tile([P, D], F32, tag="ao")
                nc.vector.tensor_scalar_mul(ao[:], att[:], w[:])
                nc.gpsimd.dma_start(
                    out_d[b, qb * P : (qb + 1) * P, D : 2 * D], ao[:]
                )

                so = opool.tile([P, S], F32, tag="so")
                nc.vector.tensor_scalar_mul(so[:, 0:512], pt[:, 0:512], w[:])
                nc.scalar.activation(
                    so[:, 512:1024], pt[:, 512:1024], AF.Copy, scale=w[:]
                )
                nc.gpsimd.dma_start(sc_d[b, qb * P : (qb + 1) * P, :], so[:])


_NC_CACHE = {}


def _get_nc():
    if "nc" not in _NC_CACHE:
        _NC_CACHE["nc"] = build_kernel()
    return _NC_CACHE["nc"]


def kernel(context, query, length):
    context = np.ascontiguousarray(np.asarray(context, dtype=np.float32))
    query = np.ascontiguousarray(np.asarray(query, dtype=np.float32))
    length = np.asarray(length).astype(np.int64)

    iot = np.arange(S)
    keymask = iot[None, :] < length[:, None]                      # [B, S]
    kbH = np.where(keymask, np.float32(0.0), NEG).astype(np.float32)
    kbH = np.ascontiguousarray(kbH.reshape(B, NT, P).transpose(0, 2, 1))
    qmH = keymask.astype(np.float32)
    qmH = np.ascontiguousarray(qmH.reshape(B, NT, P).transpose(0, 2, 1))
    id16 = np.eye(P, dtype=ml_dtypes.bfloat16)

    in_maps = []
    for c in range(NCORES):
        sl = slice(c * BPC, (c + 1) * BPC)
        in_maps.append(
            {
                "query": np.ascontiguousarray(query[sl]),
                "context": np.ascontiguousarray(context[sl]),
                "keybias": np.ascontiguousarray(kbH[sl]),
                "qmask": np.ascontiguousarray(qmH[sl]),
                "identity16": id16,
                "identity_r": np.eye(P, dtype=np.float32),
            }
        )

    nc = _get_nc()
    res = run_bass_kernel_spmd(nc, in_maps, list(range(NCORES)))
    _NC_CACHE["last_result"] = res
    out = np.concatenate([res.results[c]["out"] for c in range(NCORES)], axis=0)
    scores = np.concatenate(
        [res.results[c]["scores"] for c in range(NCORES)], axis=0
    )
    return out, scores
